# revision 1
# baseline (speedup 1.0000x reference)
"""Trainium2 Bass kernel for nn_BoxCrossCategoryLoss (8-core data-parallel).

Math: the reference loss is, per row,
    sum over 36 terms of relu(pAB[i][:,f1] + pBC[j][:,f2] - c)
where c is either pAC[k][:,1] (14 LOSS terms) or log1mexp(pAC[k][:,0])
(22 NEG terms), and p* = create_probabilities(log-volumes).  The three
int *_rel_id inputs are unused by the reference, so they are never
uploaded.

Decomposition used on-chip (per core, rows laid out as [128, NF] bf16):
  e = Exp(v)                      (ACT, fp32)
  l = Ln(1 - e)                   (ACT, scale=-1 bias=1, bf16 out)
  p-values   = v+l / l+v / v+v / l+l      (DVE tensor_tensor, bf16 2x)
  L_k = Ln(1 - P_k),  P_k = products of e / (1-e)  (DVE muls, ACT Ln)
  S = pAB + pBC               (14 sums, DVE bf16 2x)
  d = S - c                   (36 subs, DVE bf16 2x)
  relu+reduce: tensor_scalar(max,0)+accum_out (DVE 4x) or
               activation(Relu)+accum_out (ACT), split for engine balance.
Per-partition partial sums land in fp32 stats tiles, DMA'd out and
summed on host in float64.  bf16 end-to-end rel err ~4e-5 (validated).
"""

import os
import sys

import numpy as np

for _p in ("/opt/trn_rl_repo", "/root/.axon_site/_ro/trn_rl_repo"):
    if os.path.isdir(_p) and _p not in sys.path:
        sys.path.insert(0, _p)

import ml_dtypes  # noqa: E402
import concourse.bacc as bacc  # noqa: E402
from concourse import mybir, tile  # noqa: E402
from concourse.bass_utils import run_bass_kernel_spmd  # noqa: E402

BF16 = ml_dtypes.bfloat16
F32 = mybir.dt.float32
BF = mybir.dt.bfloat16
Alu = mybir.AluOpType
Act = mybir.ActivationFunctionType

N_CORES = 8
P = 128

PAIR_NAMES = ["AB", "BA", "BC", "CB", "AC", "CA"]
# Padding rows must contribute exactly zero loss: very negative AB/BC
# volumes make every S ~ -40 while c stays <= ~0, so relu(S-c) == 0.
PAD_VAL = {"AB": -20.0, "BA": -20.0, "BC": -20.0, "CB": -20.0,
           "AC": -1e-3, "CA": -1e-3}

# S_i = A[a] + B[b]  with  X[k,c] = pX[k][:, c]
S_DEFS = [
    ((0, 0), (0, 1)), ((0, 0), (2, 1)), ((1, 0), (1, 1)), ((1, 0), (2, 1)),
    ((2, 0), (0, 1)), ((2, 0), (1, 1)), ((2, 0), (2, 1)), ((2, 0), (3, 1)),
    ((0, 1), (0, 0)), ((0, 1), (2, 0)), ((1, 1), (1, 0)), ((1, 1), (2, 0)),
    ((2, 1), (2, 0)), ((3, 1), (2, 0)),
]
# 36 terms: (S index, c name);  Ck1 = pAC[k][:,1], Lk = log1mexp(pAC[k][:,0])
TERMS = [
    (0, "C01"), (1, "C01"), (2, "C11"), (3, "C11"), (4, "C01"), (5, "C11"),
    (6, "C21"), (7, "C31"), (8, "C01"), (9, "C01"), (10, "C11"), (11, "C11"),
    (12, "C21"), (13, "C31"),
    (0, "L1"), (0, "L2"), (1, "L1"), (1, "L2"), (2, "L0"), (2, "L2"),
    (3, "L0"), (3, "L2"), (4, "L1"), (4, "L2"), (5, "L0"), (5, "L2"),
    (8, "L1"), (8, "L2"), (9, "L1"), (9, "L2"), (10, "L0"), (10, "L2"),
    (11, "L0"), (11, "L2"), (7, "L2"), (13, "L2"),
]
# Engine split for the fused relu+reduce, assigned per S-group so the
# 1-3 terms of one S batch into a single slab op.  ACT groups carry 20
# term-passes, DVE groups 16 (balances the two engines).
# Cost-model sweep: ACT 15/20/26 term-passes -> 473/464/473 us; 20 is
# the balanced optimum on TRN2 (DVE relu 4x vs ACT relu 1x rates).
ACT_GROUPS = {0, 1, 4, 8, 9, 2, 7}     # S1,S2,S5,S9,S10,S3,S8 -> 20 passes
N_ACT = len(ACT_GROUPS)                 # 7 relu slots/chunk on ACT
N_DVE = 14 - N_ACT                      # 7 on DVE (16 term-passes)

A_SLOTS = [(0, 0), (1, 0), (2, 0), (0, 1), (1, 1), (2, 1), (3, 1)]


def make_chunks(nf: int) -> list[int]:
    chunks = [1344] * (nf // 1344)
    rem = nf - 1344 * len(chunks)
    if rem:
        chunks.append(rem)
    assert sum(chunks) == nf and all(c % 2 == 0 for c in chunks)
    return chunks


def _emit_pvals(nc, pool, tag_pre, F, v0, l0, v1, l1, slots):
    """p-values for one tensor pair; v*/l* keyed by col."""
    out = {}
    for (k, c) in slots:
        t = pool.tile([P, F], BF, tag=f"{tag_pre}{k}{c}")
        vv1, ll1 = v0[c], l0[c]
        vv2, ll2 = v1[c], l1[c]
        if k == 0:
            nc.vector.tensor_tensor(t[:], vv1[:], ll2[:], Alu.add)
        elif k == 1:
            nc.vector.tensor_tensor(t[:], ll1[:], vv2[:], Alu.add)
        elif k == 2:
            nc.vector.tensor_tensor(t[:], vv1[:], vv2[:], Alu.add)
        else:
            nc.vector.tensor_tensor(t[:], ll1[:], ll2[:], Alu.add)
        out[(k, c)] = t
    return out


def build_module(nf: int, chunks: list[int]):
    nchunks = len(chunks)
    nc = bacc.Bacc("TRN2", target_bir_lowering=False, debug=False,
                   enable_asserts=False, num_devices=N_CORES)
    in_aps = {}
    for X in PAIR_NAMES:
        for c in (0, 1):
            h = nc.dram_tensor(f"v_{X}{c}", [P, nf], BF, kind="ExternalInput")
            in_aps[(X, c)] = h.ap()
    out_v = nc.dram_tensor("stats_v", [P, N_DVE * nchunks], F32,
                           kind="ExternalOutput").ap()
    out_a = nc.dram_tensor("stats_a", [P, N_ACT * nchunks], F32,
                           kind="ExternalOutput").ap()

    terms_of = {}
    for ti, (si, cn) in enumerate(TERMS):
        terms_of.setdefault(si, []).append((ti, cn))

    from contextlib import ExitStack
    with tile.TileContext(nc) as tc, ExitStack() as ctx:
        vp = ctx.enter_context(tc.tile_pool(name="vp", bufs=1))
        ep = ctx.enter_context(tc.tile_pool(name="ep", bufs=4))
        lp = ctx.enter_context(tc.tile_pool(name="lp", bufs=5))
        mp = ctx.enter_context(tc.tile_pool(name="mp", bufs=3))
        pv = ctx.enter_context(tc.tile_pool(name="pv", bufs=1))
        sp = ctx.enter_context(tc.tile_pool(name="sp", bufs=4))
        Lp = ctx.enter_context(tc.tile_pool(name="Lp", bufs=1))
        dp = ctx.enter_context(tc.tile_pool(name="dp", bufs=2))
        stp = ctx.enter_context(tc.tile_pool(name="st", bufs=1))

        stats_v = stp.tile([P, N_DVE * nchunks], F32, tag="stv")
        stats_a = stp.tile([P, N_ACT * nchunks], F32, tag="sta")

        f0 = 0
        for k, F in enumerate(chunks):
            # ---- load all 12 column tiles for this chunk (AC pair first)
            v = {}
            for X in ["AC", "CA", "AB", "BA", "BC", "CB"]:
                for c in (0, 1):
                    t = vp.tile([P, F], BF, tag=f"v{X}{c}")
                    nc.sync.dma_start(t[:], in_aps[(X, c)][:, f0:f0 + F])
                    v[(X, c)] = t

            def exp_of(X, c):
                e = ep.tile([P, F], F32, tag="e")
                nc.scalar.activation(e[:], v[(X, c)][:], Act.Exp)
                return e

            def ln1m(src, dst_tag, pool):
                t = pool.tile([P, F], BF, tag=dst_tag)
                nc.scalar.activation(t[:], src[:], Act.Ln, bias=1.0, scale=-1.0)
                return t

            # ---- pair AC: e, l (col1 only), m/P products, L_k, C p-values
            eAC0, eCA0 = exp_of("AC", 0), exp_of("CA", 0)
            eAC1, eCA1 = exp_of("AC", 1), exp_of("CA", 1)
            lAC1 = ln1m(eAC1, "l", lp)
            lCA1 = ln1m(eCA1, "l", lp)
            mAC = mp.tile([P, F], F32, tag="mP")
            nc.vector.tensor_scalar(mAC[:], eAC0[:], -1.0, 1.0, Alu.mult, Alu.add)
            mCA = mp.tile([P, F], F32, tag="mP")
            nc.vector.tensor_scalar(mCA[:], eCA0[:], -1.0, 1.0, Alu.mult, Alu.add)
            L = {}
            for j, (x, y) in enumerate([(eAC0, mCA), (mAC, eCA0), (eAC0, eCA0)]):
                Pj = mp.tile([P, F], F32, tag="mP")
                nc.vector.tensor_tensor(Pj[:], x[:], y[:], Alu.mult)
                L[f"L{j}"] = ln1m(Pj, f"L{j}", Lp)
            Cvals = _emit_pvals(nc, pv, "C", F,
                                {1: v[("AC", 1)]}, {1: lAC1},
                                {1: v[("CA", 1)]}, {1: lCA1},
                                [(0, 1), (1, 1), (2, 1), (3, 1)])

            # ---- pair AB -> A p-values
            eAB0, eBA0 = exp_of("AB", 0), exp_of("BA", 0)
            eAB1, eBA1 = exp_of("AB", 1), exp_of("BA", 1)
            lAB = {0: ln1m(eAB0, "l", lp), 1: ln1m(eAB1, "l", lp)}
            lBA = {0: ln1m(eBA0, "l", lp), 1: ln1m(eBA1, "l", lp)}
            Avals = _emit_pvals(nc, pv, "A", F,
                                {0: v[("AB", 0)], 1: v[("AB", 1)]}, lAB,
                                {0: v[("BA", 0)], 1: v[("BA", 1)]}, lBA,
                                A_SLOTS)

            # ---- pair BC -> B p-values
            eBC0, eCB0 = exp_of("BC", 0), exp_of("CB", 0)
            eBC1, eCB1 = exp_of("BC", 1), exp_of("CB", 1)
            lBC = {0: ln1m(eBC0, "l", lp), 1: ln1m(eBC1, "l", lp)}
            lCB = {0: ln1m(eCB0, "l", lp), 1: ln1m(eCB1, "l", lp)}
            Bvals = _emit_pvals(nc, pv, "B", F,
                                {0: v[("BC", 0)], 1: v[("BC", 1)]}, lBC,
                                {0: v[("CB", 0)], 1: v[("CB", 1)]}, lCB,
                                A_SLOTS)

            cmap = {f"C{kk}1": Cvals[(kk, 1)] for kk in range(4)}
            cmap.update(L)
            # TERMS c-names use "C01" == pAC[0][:,1]
            cmap = {"C01": cmap["C01"], "C11": cmap["C11"],
                    "C21": cmap["C21"], "C31": cmap["C31"],
                    "L0": cmap["L0"], "L1": cmap["L1"], "L2": cmap["L2"]}

            # ---- S sums + 36 terms; the subs of one S write a contiguous
            # d-slab so its relu+reduce is ONE fused op over the slab.
            na = nv = 0
            for si, (a, b) in enumerate(S_DEFS):
                S = sp.tile([P, F], BF, tag="S")
                nc.vector.tensor_tensor(S[:], Avals[a][:], Bvals[b][:], Alu.add)
                terms = terms_of[si]
                nt = len(terms)
                d = dp.tile([P, nt * F], BF, tag="d")
                for j, (ti, cn) in enumerate(terms):
                    nc.vector.tensor_tensor(d[:, j * F:(j + 1) * F], S[:],
                                            cmap[cn][:], Alu.subtract)
                r = dp.tile([P, nt * F], BF, tag="r")
                if si in ACT_GROUPS:
                    slot = stats_a[:, k * N_ACT + na: k * N_ACT + na + 1]
                    nc.scalar.activation(r[:], d[:], Act.Relu, accum_out=slot)
                    na += 1
                else:
                    slot = stats_v[:, k * N_DVE + nv: k * N_DVE + nv + 1]
                    nc.vector.tensor_scalar(r[:], d[:], 0.0, None, Alu.max,
                                            Alu.add, accum_out=slot)
                    nv += 1
            assert na == N_ACT and nv == N_DVE
            f0 += F

        nc.sync.dma_start(out_v, stats_v[:])
        nc.sync.dma_start(out_a, stats_a[:])

    nc.compile()
    return nc


_CACHE = {}


def _get_module(nf, chunks):
    key = (nf, tuple(chunks))
    if key not in _CACHE:
        _CACHE[key] = build_module(nf, chunks)
    return _CACHE[key]


LAST_RESULTS = None  # BassKernelResults of the most recent run (for profiling)


def kernel(**inputs) -> np.ndarray:
    global LAST_RESULTS
    vols = {X: np.asarray(inputs["vol_" + X]) for X in PAIR_NAMES}
    n_rows = vols["AB"].shape[0]
    # rows per core laid out [128, nf]; nf even for DVE packed modes
    nf = -(-n_rows // (N_CORES * P))
    nf += nf % 2
    nf = max(nf, 160)
    # round up so chunking stays regular (multiples of 32 keep DMA tidy)
    nf = -(-nf // 32) * 32
    chunks = make_chunks(nf)
    total_rows = N_CORES * P * nf

    in_maps = [dict() for _ in range(N_CORES)]
    for X in PAIR_NAMES:
        a = vols[X].astype(np.float32, copy=False)
        for c in (0, 1):
            col = np.full(total_rows, PAD_VAL[X], dtype=np.float32)
            col[:n_rows] = a[:, c]
            colb = col.astype(BF16).reshape(N_CORES, P, nf)
            for core in range(N_CORES):
                in_maps[core][f"v_{X}{c}"] = np.ascontiguousarray(colb[core])

    nc = _get_module(nf, chunks)
    # NTFF tracing needs antenv.axon_hooks, absent in most axon client
    # environments; force it off so a stray BASS_TRACE can't crash the run.
    trace = bool(os.environ.get("BASS_TRACE"))
    if trace:
        try:
            from antenv import axon_hooks  # noqa: F401
        except ImportError:
            trace = False
    if not trace:
        os.environ["BASS_NEVER_TRACE"] = "1"
    res = run_bass_kernel_spmd(nc, in_maps, core_ids=list(range(N_CORES)),
                               trace=trace)
    LAST_RESULTS = res
    total = np.float64(0.0)
    for om in res.results:
        total += om["stats_v"].astype(np.float64).sum()
        total += om["stats_a"].astype(np.float64).sum()
    return np.asarray(total, dtype=np.float32)


if __name__ == "__main__":
    # quick smoke test on small random data
    rng = np.random.default_rng(0)
    n = 100_000
    ins = {}
    for X in PAIR_NAMES:
        u = rng.uniform(1e-6, 1 - 1e-6, size=(n, 2)).astype(np.float32)
        ins["vol_" + X] = np.log(u)
    for nm in ("xy_rel_id", "yz_rel_id", "xz_rel_id"):
        ins[nm] = rng.integers(0, 2, size=(n, 2)).astype(np.int32)
    print("kernel:", kernel(**ins))



# revision 4
# speedup vs baseline: 1.2510x; 1.2510x over previous
"""Trainium2 Bass kernel for nn_BoxCrossCategoryLoss (8-core data-parallel).

Math per row (36 terms): relu(pAB[i][:,f1] + pBC[j][:,f2] - c) where c is
pAC[k][:,1] (14 terms) or log1mexp(pAC[k][:,0]) (22 terms).  The int
*_rel_id inputs are unused by the reference and never uploaded.

Strategy (vs the previous kernel): loss = sum of relu over 7 "c-groups",
each computed as one broadcast tensor_tensor subtract over a permuted
S-slab followed by one fused relu+accumulate.  Work is LP-balanced over
three engines:
  ACT : 12 Exp + 13 Ln (the irreducible transcendentals) + relu of the
        two biggest groups (17 col-passes) with accum_out.
  DVE : all p-value/S adds (tensor_tensor bf16, 2x mode), subs for 6
        groups, relu+acc (tensor_scalar 4x) for 5 groups.
  Pool (gpsimd): the L2-group subtract (12 col-passes).
All Ln ops use bias = 1+2^-12 so log arguments stay > 0 (no -inf/NaN can
reach an accumulator); bf16(exp)=1.0 rows yield relu()=0 exactly as the
true term is ~0 there.  Per-partition partial sums land in fp32 stats,
DMA'd out and reduced on host in float64.
"""

import os
import sys

import numpy as np

for _p in ("/opt/trn_rl_repo", "/root/.axon_site/_ro/trn_rl_repo"):
    if os.path.isdir(_p) and _p not in sys.path:
        sys.path.insert(0, _p)

import ml_dtypes  # noqa: E402
import concourse.bacc as bacc  # noqa: E402
from concourse import mybir, tile  # noqa: E402
from concourse.bass_utils import run_bass_kernel_spmd  # noqa: E402

BF16 = ml_dtypes.bfloat16
F32 = mybir.dt.float32
BF = mybir.dt.bfloat16
Alu = mybir.AluOpType
Act = mybir.ActivationFunctionType

N_CORES = 8
P = 128
LNBIAS = 1.000244140625  # 1 + 2^-12: keeps every Ln argument positive

# Input column order inside the packed per-chunk slab.
# 0:AB0 1:AB1 2:BA0 3:BA1 4:BC0 5:BC1 6:CB0 7:CB1 8:AC1 9:CA1 10:AC0 11:CA0
COLS = ["AB0", "AB1", "BA0", "BA1", "BC0", "BC1", "CB0", "CB1",
        "AC1", "CA1", "AC0", "CA0"]
PAD_VAL = {"AB": -20.0, "BA": -20.0, "BC": -20.0, "CB": -20.0,
           "AC": -1e-3, "CA": -1e-3}

# A/B slab layout: [X00 X01 X10 X11 X20 X21 X31] (index 2k+c, X31 at 6)
# S-slab permuted order (positions chosen so every c-group is contiguous):
#   pos:  0  1  2  3  4 |  5  6  7  8  9 | 10 11 | 12 13
#   S  : S0 S1 S4 S8 S9 | S2 S3 S5 S10 S11 | S7 S13 | S6 S12
# S_i = A[a_i] + B[b_i]  (slab indices into A/B slabs)
S_AT_POS = [  # (A-slab idx, B-slab idx) per S-slab position
    (0, 1), (0, 5), (4, 1), (1, 0), (1, 4),      # S0 S1 S4 S8 S9
    (2, 3), (2, 5), (4, 3), (3, 2), (3, 4),      # S2 S3 S5 S10 S11
    (4, 6), (6, 4),                              # S7 S13
    (4, 5), (5, 4),                              # S6 S12
]
# groups: (name, slab_start_pos, n_terms, c_source)
# c_source: ('C', k) -> C-slab col k;  ('L01', k) -> LP01 col k; ('L2',)
G_C01 = ("C01", 0, 5, ("C", 0))
G_L1 = ("L1", 0, 5, ("L01", 1))
G_C11 = ("C11", 5, 5, ("C", 1))
G_L0 = ("L0", 5, 5, ("L01", 0))
G_C31 = ("C31", 10, 2, ("C", 3))
G_C21 = ("C21", 12, 2, ("C", 2))
G_L2 = ("L2", 0, 12, ("L2",))

DVE_GROUPS = [G_L1, G_C11, G_L0, G_C31, G_C21]   # sub+relu+acc on DVE
ACT_RELU_GROUPS = [G_C01, G_L2]                   # sub on DVE/Pool, relu on ACT


def build_module(F: int, rounds: int):
    nf = F * rounds
    nc = bacc.Bacc("TRN2", target_bir_lowering=False, debug=False,
                   enable_asserts=False, num_devices=N_CORES)
    # register the Ln bias constant (only 0.0/1.0 are pre-registered)
    _ct = nc.alloc_sbuf_tensor(f"const-f32-{LNBIAS}", [P, 1], F32)
    nc.gpsimd.memset(_ct.ap(), LNBIAS)
    nc.const_aps.aps[(F32, LNBIAS)] = _ct.ap()
    nc.all_engine_barrier()

    vin = nc.dram_tensor("vin", [P, 12 * nf], BF, kind="ExternalInput").ap()
    n_dve = len(DVE_GROUPS)
    n_act = len(ACT_RELU_GROUPS)
    out_d = nc.dram_tensor("stats_d", [P, n_dve * rounds], F32,
                           kind="ExternalOutput").ap()
    out_a = nc.dram_tensor("stats_a", [P, n_act * rounds], F32,
                           kind="ExternalOutput").ap()

    from contextlib import ExitStack
    with tile.TileContext(nc) as tc, ExitStack() as ctx:
        p1 = ctx.enter_context(tc.tile_pool(name="p1", bufs=1))
        p2 = ctx.enter_context(tc.tile_pool(name="p2", bufs=2))
        stp = ctx.enter_context(tc.tile_pool(name="st", bufs=1))

        st_d = stp.tile([P, n_dve * rounds], F32, tag="std")
        st_a = stp.tile([P, n_act * rounds], F32, tag="sta")

        def bcast(c_ap, n):
            return c_ap[:, None, :].broadcast_to([P, n, F])

        def view3(t_ap, n):
            return t_ap.rearrange("p (a b) -> p a b", a=n)

        pend = None  # deferred ACT relus from the previous round

        for k in range(rounds):
            IN = p2.tile([P, 12 * F], BF, tag="in")
            nc.sync.dma_start(IN[:], vin[:, k * 12 * F:(k + 1) * 12 * F])

            # ---------------- ACT: transcendentals ----------------
            EL = p2.tile([P, 2 * F], BF, tag="el")     # exp(AC0), exp(CA0)
            nc.scalar.activation(EL[:], IN[:, 10 * F:12 * F], Act.Exp)
            E8 = p1.tile([P, 8 * F], BF, tag="e8")
            nc.scalar.activation(E8[:], IN[:, 0:8 * F], Act.Exp)
            L8 = p1.tile([P, 8 * F], BF, tag="l8")
            nc.scalar.activation(L8[:], E8[:], Act.Ln, bias=LNBIAS, scale=-1.0)
            EC = p1.tile([P, 2 * F], F32, tag="ec")    # exp(AC1), exp(CA1)
            nc.scalar.activation(EC[:], IN[:, 8 * F:10 * F], Act.Exp)
            # EC is fp32 and strictly < 1, so bias=1.0 is exact and safe here
            LC = p2.tile([P, 2 * F], BF, tag="lc")     # lAC1, lCA1
            nc.scalar.activation(LC[:], EC[:], Act.Ln, bias=1.0, scale=-1.0)

            # ---------------- DVE/Pool: products for L terms ----------------
            M = p1.tile([P, 2 * F], BF, tag="m")       # 1-exp
            nc.vector.tensor_scalar(M[:], EL[:], -1.0, 1.0, Alu.mult, Alu.add)
            Pp = p2.tile([P, 2 * F], BF, tag="pp")     # P0, P1
            nc.vector.tensor_tensor(Pp[:, 0:F], EL[:, 0:F], M[:, F:2 * F],
                                    Alu.mult)
            nc.vector.tensor_tensor(Pp[:, F:2 * F], M[:, 0:F], EL[:, F:2 * F],
                                    Alu.mult)
            P2t = p2.tile([P, F], BF, tag="p2")        # P2
            nc.vector.tensor_tensor(P2t[:], EL[:, 0:F], EL[:, F:2 * F],
                                    Alu.mult)

            # ---------------- ACT: L-values ----------------
            LP01 = p2.tile([P, 2 * F], BF, tag="lp01")  # L0, L1
            nc.scalar.activation(LP01[:], Pp[:], Act.Ln, bias=LNBIAS,
                                 scale=-1.0)
            LL2 = p2.tile([P, F], BF, tag="ll2")        # L2
            nc.scalar.activation(LL2[:], P2t[:], Act.Ln, bias=LNBIAS,
                                 scale=-1.0)

            # ---------------- ACT: deferred relus from round k-1 ----------
            if pend is not None:
                _emit_act_relus(nc, *pend)
                pend = None

            # ---------------- DVE: p-value adds ----------------
            A = p1.tile([P, 7 * F], BF, tag="A")
            B = p1.tile([P, 7 * F], BF, tag="B")
            for X, ofs in ((A, 0), (B, 4 * F)):
                v = IN[:, ofs:ofs + 4 * F]      # [v1c0 v1c1 v2c0 v2c1]
                l = L8[:, ofs:ofs + 4 * F]
                nc.vector.tensor_tensor(X[:, 0:2 * F], v[:, 0:2 * F],
                                        l[:, 2 * F:4 * F], Alu.add)  # X0c
                nc.vector.tensor_tensor(X[:, 2 * F:4 * F], l[:, 0:2 * F],
                                        v[:, 2 * F:4 * F], Alu.add)  # X1c
                nc.vector.tensor_tensor(X[:, 4 * F:6 * F], v[:, 0:2 * F],
                                        v[:, 2 * F:4 * F], Alu.add)  # X2c
                nc.vector.tensor_tensor(X[:, 6 * F:7 * F], l[:, F:2 * F],
                                        l[:, 3 * F:4 * F], Alu.add)  # X31
            C = p1.tile([P, 4 * F], BF, tag="C")       # C01 C11 C21 C31
            vA1, vC1 = IN[:, 8 * F:9 * F], IN[:, 9 * F:10 * F]
            lA1, lC1 = LC[:, 0:F], LC[:, F:2 * F]
            nc.vector.tensor_tensor(C[:, 0:F], vA1, lC1, Alu.add)
            nc.vector.tensor_tensor(C[:, F:2 * F], lA1, vC1, Alu.add)
            nc.vector.tensor_tensor(C[:, 2 * F:3 * F], vA1, vC1, Alu.add)
            nc.vector.tensor_tensor(C[:, 3 * F:4 * F], lA1, lC1, Alu.add)

            # ---------------- DVE: S-slab ----------------
            S = p1.tile([P, 14 * F], BF, tag="S")
            for pos, (ai, bi) in enumerate(S_AT_POS):
                nc.vector.tensor_tensor(S[:, pos * F:(pos + 1) * F],
                                        A[:, ai * F:(ai + 1) * F],
                                        B[:, bi * F:(bi + 1) * F], Alu.add)

            def c_ap(src):
                if src[0] == "C":
                    return C[:, src[1] * F:(src[1] + 1) * F]
                if src[0] == "L01":
                    return LP01[:, src[1] * F:(src[1] + 1) * F]
                return LL2[:]

            # ---------------- DVE groups: sub + relu + acc ----------------
            dD = p1.tile([P, 5 * F], BF, tag="dD")
            for gi, (name, pos, nt, src) in enumerate(DVE_GROUPS):
                d = dD[:, 0:nt * F]
                nc.vector.tensor_tensor(view3(d, nt),
                                        view3(S[:, pos * F:(pos + nt) * F], nt),
                                        bcast(c_ap(src), nt), Alu.subtract)
                nc.vector.tensor_scalar(d, d, 0.0, None, Alu.max, Alu.add,
                                        accum_out=st_d[:, k * n_dve + gi:
                                                       k * n_dve + gi + 1])

            # ---------------- ACT-relu groups: subs (DVE g1, Pool g7) ------
            dA1 = p1.tile([P, 5 * F], BF, tag="dA1")   # C01 group
            name, pos, nt, src = G_C01
            nc.vector.tensor_tensor(view3(dA1[:], nt),
                                    view3(S[:, pos * F:(pos + nt) * F], nt),
                                    bcast(c_ap(src), nt), Alu.subtract)
            dA7 = p1.tile([P, 12 * F], BF, tag="dA7")  # L2 group on Pool
            name, pos, nt, src = G_L2
            nc.gpsimd.tensor_tensor(view3(dA7[:], nt),
                                    view3(S[:, pos * F:(pos + nt) * F], nt),
                                    bcast(c_ap(src), nt), Alu.subtract)
            pend = (st_a, k, n_act, dA1, dA7)

        # tail: relus of the last round
        _emit_act_relus(nc, *pend)

        nc.sync.dma_start(out_d, st_d[:])
        nc.sync.dma_start(out_a, st_a[:])

    nc.compile()
    return nc


def _emit_act_relus(nc, st_a, k, n_act, dA1, dA7):
    nc.scalar.activation(dA1[:], dA1[:], Act.Relu,
                         accum_out=st_a[:, k * n_act:k * n_act + 1])
    nc.scalar.activation(dA7[:], dA7[:], Act.Relu,
                         accum_out=st_a[:, k * n_act + 1:k * n_act + 2])


_CACHE = {}


def _get_module(F, rounds):
    key = (F, rounds)
    if key not in _CACHE:
        _CACHE[key] = build_module(F, rounds)
    return _CACHE[key]


LAST_RESULTS = None  # BassKernelResults of the most recent run (for profiling)


def _plan(n_rows):
    per_core = -(-n_rows // (N_CORES * P))  # free elems per partition
    for F, rounds in ((872, 9), (784, 10), (712, 11), (656, 12), (560, 14)):
        if F * rounds >= per_core:
            best = (F, rounds)
    # pick the smallest nf that covers; iterate ascending instead
    cand = [(F, r) for (F, r) in ((872, 9), (784, 10), (712, 11), (656, 12),
                                  (560, 14)) if F * r >= per_core]
    cand.sort(key=lambda fr: fr[0] * fr[1])
    if cand:
        return cand[0]
    rounds = -(-per_core // 872)
    return 872, rounds


def kernel(**inputs) -> np.ndarray:
    global LAST_RESULTS
    vols = {X: np.asarray(inputs["vol_" + X]) for X in
            ["AB", "BA", "BC", "CB", "AC", "CA"]}
    n_rows = vols["AB"].shape[0]
    F, rounds = _plan(n_rows)
    nf = F * rounds
    total_rows = N_CORES * P * nf

    # full column stack in the packed order, padded
    cols = np.empty((12, total_rows), dtype=BF16)
    for j, name in enumerate(COLS):
        X, c = name[:2], int(name[2])
        col = np.full(total_rows, PAD_VAL[X], dtype=np.float32)
        col[:n_rows] = vols[X][:, c].astype(np.float32, copy=False)
        cols[j] = col.astype(BF16)
    # [12, cores, P, rounds, F] -> per core [P, rounds, 12, F]
    cview = cols.reshape(12, N_CORES, P, rounds, F)
    in_maps = []
    for core in range(N_CORES):
        pk = np.ascontiguousarray(
            cview[:, core].transpose(1, 2, 0, 3)).reshape(P, 12 * nf)
        in_maps.append({"vin": pk})

    nc = _get_module(F, rounds)
    trace = bool(os.environ.get("BASS_TRACE"))
    if trace:
        try:
            from antenv import axon_hooks  # noqa: F401
        except ImportError:
            trace = False
    if not trace:
        os.environ["BASS_NEVER_TRACE"] = "1"
    res = run_bass_kernel_spmd(nc, in_maps, core_ids=list(range(N_CORES)),
                               trace=trace)
    LAST_RESULTS = res
    total = np.float64(0.0)
    for om in res.results:
        total += om["stats_d"].astype(np.float64).sum()
        total += om["stats_a"].astype(np.float64).sum()
    return np.asarray(total, dtype=np.float32)


if __name__ == "__main__":
    rng = np.random.default_rng(0)
    n = 100_000
    ins = {}
    for X in ["AB", "BA", "BC", "CB", "AC", "CA"]:
        u = rng.uniform(1e-6, 1 - 1e-6, size=(n, 2)).astype(np.float32)
        ins["vol_" + X] = np.log(u)
    for nm in ("xy_rel_id", "yz_rel_id", "xz_rel_id"):
        ins[nm] = rng.integers(0, 2, size=(n, 2)).astype(np.int32)
    print("kernel:", kernel(**ins))

    # reference check on host
    def log1mexp(x):
        return np.log1p(-np.exp(x))
    DM = {0: 0, 1: 0, 2: 0, 3: 0, 4: 1, 5: 1, 6: 1, 7: 1}
    LR = [(0, 4, 4), (0, 6, 4), (1, 5, 5), (1, 6, 5), (2, 4, 4), (2, 5, 5),
          (2, 6, 6), (2, 7, 7), (4, 0, 4), (4, 2, 4), (5, 1, 5), (5, 2, 5),
          (6, 2, 6), (7, 2, 7)]
    NLR = [(0, 4, 1), (0, 4, 2), (0, 6, 1), (0, 6, 2), (1, 5, 0), (1, 5, 2),
           (1, 6, 0), (1, 6, 2), (2, 4, 1), (2, 4, 2), (2, 5, 0), (2, 5, 2),
           (4, 0, 1), (4, 0, 2), (4, 2, 1), (4, 2, 2), (5, 1, 0), (5, 1, 2),
           (5, 2, 0), (5, 2, 2), (2, 7, 2), (7, 2, 2)]

    def probs(v1, v2):
        l1, l2 = log1mexp(v1), log1mexp(v2)
        return [v1 + l2, l1 + v2, v1 + v2, l1 + l2]
    pAB = probs(ins["vol_AB"], ins["vol_BA"])
    pBC = probs(ins["vol_BC"], ins["vol_CB"])
    pAC = probs(ins["vol_AC"], ins["vol_CA"])
    loss = 0.0
    for xy, yz, xz in LR:
        t = (pAB[xy % 4][:, DM[xy]] + pBC[yz % 4][:, DM[yz]]
             - pAC[xz % 4][:, DM[xz]])
        loss += np.maximum(0, t).sum(dtype=np.float64)
    for xy, yz, xz in NLR:
        t = (pAB[xy % 4][:, DM[xy]] + pBC[yz % 4][:, DM[yz]]
             - log1mexp(pAC[xz % 4][:, DM[xz]]))
        loss += np.maximum(0, t).sum(dtype=np.float64)
    print("expected:", loss)
